# revision 10
# baseline (speedup 1.0000x reference)
"""GCN encoder (6 GCNConv layers, shared residual block) on 8 trn2 NeuronCores.

Strategy:
  - Nodes are relabeled (load-balanced bin packing by in-degree) and padded to
    NPAD = CORES * NB * 128. Core c owns the contiguous row range
    [c*PER, (c+1)*PER)  (PER = NB*128), i.e. NB blocks of 128 dst nodes.
  - Per conv: each core computes G' = dinv * (H @ W) for its own rows (dense
    GEMM on the PE), casts to bf16, AllGathers G' so every core holds the full
    [NPAD, ch] table in local DRAM, then aggregates messages for its dst
    blocks: dma_gather pulls 128 source rows per chunk into SBUF in matmul
    layout, and a one-hot "scatter matrix" M (stationary, built on host)
    scatter-adds them into a [128 dst, ch] PSUM tile:
        psum = sum_c  M_c.T @ Grows_c          (PE matmul accumulation)
    Self-loops and the bias are folded in as extra edges (bias edge gathers a
    bias row appended to the G' table, with M column values 1/dinv_dst).
    Postscale by dinv_dst + ReLU is a single ScalarE activation; the residual
    add runs on VectorE against the transposed f32 H state (PE transposes).
  - dma_gather indices are int16, so the gather is split into two windows of
    the node table: window A = rows [0, 32768), window B = rows
    [NPAD+1-32768, NPAD+1). Edges with src in the overlap can go to either
    call; the host balances the two chunk lists per block.
"""

import sys

sys.path.insert(0, "/opt/trn_rl_repo")

from dataclasses import dataclass

import numpy as np

import concourse.bass as bass
import concourse.bacc as bacc
import concourse.mybir as mybir
from concourse.bass_utils import run_bass_kernel_spmd
from concourse.tile import TileContext

BF16 = mybir.dt.bfloat16
F32 = mybir.dt.float32
I16 = mybir.dt.int16
NP_BF16 = mybir.dt.np(BF16)


@dataclass
class GCNConfig:
    n_nodes: int = 50000
    cores: int = 8
    nb: int = 49          # dst blocks per core
    in_ch: int = 512
    h1: int = 512
    h2: int = 256
    depth: int = 4        # residual reuses of Wr
    # derived
    @property
    def per(self):
        return self.nb * 128

    @property
    def npad(self):
        return self.cores * self.per

    @property
    def nblk(self):
        return self.cores * self.nb


@dataclass
class Prepped:
    cfg: GCNConfig
    ca: int               # chunks per block gathered from window A
    cb: int               # chunks per block gathered from window B
    wa: int               # window A row count (in_ap rows), idx < wa
    b0: int               # window B first row;  idx = src - b0
    newid: np.ndarray     # [n_nodes] -> padded id
    per_core: list        # per core dict: Mst, idxA, idxB, dinv
    host_ms: float = 0.0


def preprocess(cfg: GCNConfig, edge_index: np.ndarray) -> Prepped:
    import time

    t0 = time.time()
    N, NPAD, NBLK = cfg.n_nodes, cfg.npad, cfg.nblk
    src = np.asarray(edge_index[0], dtype=np.int64)
    dst = np.asarray(edge_index[1], dtype=np.int64)
    deg = np.bincount(dst, minlength=N).astype(np.int64) + 1  # + self loop
    dinv = (1.0 / np.sqrt(deg)).astype(np.float32)

    # ---- load-balanced node relabeling: snake-deal nodes (sorted by degree
    # desc) into NBLK blocks of <=128 slots; block sums come out ~equal.
    order = np.argsort(-deg, kind="stable")
    n_rounds = -(-N // NBLK)
    blk_of = np.empty(N, dtype=np.int64)
    slot_of = np.empty(N, dtype=np.int64)
    pos = np.arange(N)
    r = pos // NBLK
    j = pos % NBLK
    fwd = (r % 2) == 0
    b = np.where(fwd, j, NBLK - 1 - j)
    blk_of[order] = b
    slot_of[order] = r
    assert n_rounds <= 128
    newid = blk_of * 128 + slot_of
    # dinv per padded id (pads -> 1.0)
    dinv_pad = np.ones(NPAD, dtype=np.float32)
    dinv_pad[newid] = dinv
    real_pad = np.zeros(NPAD, dtype=bool)
    real_pad[newid] = True

    # ---- edge lists in new ids, plus self loops
    es = newid[src]
    ed = newid[dst]
    es = np.concatenate([es, newid])          # self loops
    ed = np.concatenate([ed, newid])
    eb = ed >> 7                              # dst block
    eslot = ed & 127

    # sort edges by (block, src) so each block's list is src-ascending
    sort_key = eb * (NPAD + 1) + es
    so = np.argsort(sort_key, kind="stable")
    es, eb, eslot = es[so], eb[so], eslot[so]

    cnt = np.bincount(eb, minlength=NBLK)     # real+self edges per block
    tmax = int(cnt.max())
    # window geometry
    half = 32768
    wa = min(half, NPAD)                      # window A rows [0, wa)
    b0 = max(0, NPAD + 1 - half)              # window B rows [b0, NPAD+1)
    n_low = np.bincount(eb[es < b0], minlength=NBLK) if b0 > 0 else np.zeros(NBLK, int)
    n_high = np.bincount(eb[es >= wa], minlength=NBLK) if wa < NPAD else np.zeros(NBLK, int)
    ct = -(-(tmax + 1) // 128)                # +1 bias edge
    ca_lo = -(-int(n_low.max()) // 128) if b0 > 0 else 0
    cb_lo = -(-(int(n_high.max()) + 1) // 128)
    while ca_lo + cb_lo > ct:
        ct += 1
    ca = min(max(ca_lo, ct - cb_lo, ct // 2), ct - cb_lo)
    cb = ct - ca
    assert ca * 128 >= n_low.max() and cb * 128 >= n_high.max() + 1

    # per-block split index into the src-sorted edge list
    starts = np.zeros(NBLK + 1, dtype=np.int64)
    np.cumsum(cnt, out=starts[1:])
    n_lowmid = np.bincount(eb[es < wa], minlength=NBLK)
    split = np.minimum(ca * 128, n_lowmid)    # first `split` edges -> call A
    nb_b = cnt - split                        # B real edges (bias appended after)
    assert (nb_b + 1 <= cb * 128).all()

    # position of each edge within its block
    epos = np.arange(len(es)) - starts[eb]
    in_a = epos < split[eb]
    posA = epos
    posB = epos - split[eb]

    # chunk index within block (A chunks [0,ca), B chunks [ca, ct)), edge slot
    echunk = np.where(in_a, posA >> 7, ca + (posB >> 7))
    erow = np.where(in_a, posA & 127, posB & 127)

    # ---- per-core arrays
    core_of = eb // cfg.nb
    bl = eb % cfg.nb                          # block local index
    per_core = []
    ca8, cb8 = ca * 8, cb * 8
    for c in range(cfg.cores):
        m = core_of == c
        ces, cbl, cch, cer, cslot = es[m], bl[m], echunk[m], erow[m], eslot[m]
        cina = in_a[m]
        # scatter matrix M: [128, nb*ct*128] ; M[erow, (bl*ct+ch)*128 + slot] = 1
        M = np.zeros((128, cfg.nb * ct * 128), dtype=np.float32)
        M[cer, (cbl * ct + cch) * 128 + cslot] = 1.0
        # idx lists: [nb, ca*128] / [nb, cb*128] (pad 0)
        idxA = np.zeros((cfg.nb, ca * 128), dtype=np.int64)
        idxB = np.zeros((cfg.nb, cb * 128), dtype=np.int64)
        idxA[cbl[cina], (cch[cina] * 128 + cer[cina])] = ces[cina]
        mB = ~cina
        idxB[cbl[mB], ((cch[mB] - ca) * 128 + cer[mB])] = ces[mB] - b0
        # bias edge per block: B position nb_b[block]
        gblk = c * cfg.nb + np.arange(cfg.nb)
        bpos = nb_b[gblk]
        bch, brow = ca + (bpos >> 7), bpos & 127
        idxB[np.arange(cfg.nb), (bch - ca) * 128 + brow] = (NPAD - b0)
        # M row for bias edge: 1/dinv over real slots of the block
        dv = dinv_pad.reshape(NBLK, 128)[gblk]           # [nb, 128]
        rl = real_pad.reshape(NBLK, 128)[gblk]
        bias_w = np.where(rl, 1.0 / dv, 0.0).astype(np.float32)
        cols = ((np.arange(cfg.nb) * ct + bch) * 128)[:, None] + np.arange(128)[None, :]
        M[brow[:, None].repeat(128, 1), cols] = bias_w
        assert idxA.min() >= 0 and idxA.max() < wa
        assert idxB.min() >= 0 and idxB.max() < NPAD + 1 - b0
        # wrap idx to [128, nb*ca8] int16 layout: idx i at [i%16, i//16], tiled x8
        def wrap(a, c8):
            w = a.reshape(cfg.nb, c8, 16).transpose(2, 0, 1).reshape(16, cfg.nb * c8)
            return np.tile(w, (8, 1)).astype(np.int16)
        per_core.append(
            dict(
                Mst=M.astype(NP_BF16),
                idxA=wrap(idxA, ca8),
                idxB=wrap(idxB, cb8),
                dinv=dinv_pad.reshape(NBLK, 128)[gblk].T.copy(),  # [128, nb]
            )
        )
    return Prepped(cfg, ca, cb, wa, b0, newid, per_core, host_ms=(time.time() - t0) * 1e3)


def build_program(p: Prepped):
    cfg = p.cfg
    NB, PER, NPAD = cfg.nb, cfg.per, cfg.npad
    CA, CB, CT = p.ca, p.cb, p.ca + p.cb
    CH = [cfg.h1] * 5 + [cfg.h2]
    RELU = [True] * 5 + [False]
    RES = [False] + [True] * 4 + [False]
    WNAME = ["W1c"] + ["Wrc"] * 4 + ["Wxc"]
    BIDX = [0] + [1] * 4 + [2]
    KCH = cfg.in_ch // 128  # = 4

    nc = bacc.Bacc(num_devices=cfg.cores)
    xT_d = nc.dram_tensor("xT", [cfg.in_ch, PER], BF16, kind="ExternalInput")
    Mst_d = nc.dram_tensor("Mst", [128, NB * CT * 128], BF16, kind="ExternalInput")
    idxA_d = nc.dram_tensor("idxA", [128, NB * CA * 8], I16, kind="ExternalInput")
    idxB_d = nc.dram_tensor("idxB", [128, NB * CB * 8], I16, kind="ExternalInput")
    dinv_d = nc.dram_tensor("dinv", [128, NB], F32, kind="ExternalInput")
    ident_d = nc.dram_tensor("ident", [128, 128], F32, kind="ExternalInput")
    bias_d = nc.dram_tensor("biases", [3, cfg.h1], BF16, kind="ExternalInput")
    w_d = {
        "W1c": nc.dram_tensor("W1c", [128, KCH * cfg.h1], BF16, kind="ExternalInput"),
        "Wrc": nc.dram_tensor("Wrc", [128, KCH * cfg.h1], BF16, kind="ExternalInput"),
        "Wxc": nc.dram_tensor("Wxc", [128, KCH * cfg.h2], BF16, kind="ExternalInput"),
    }
    out_d = nc.dram_tensor("out", [PER, cfg.h2], F32, kind="ExternalOutput")
    rg = [list(range(cfg.cores))]

    with TileContext(nc) as tc:
        frees = []

        def persist(shape, dtype, name, space="SBUF"):
            t, f = tc.tile(shape, dtype, name=name, space=space)
            frees.append(f)
            return t

        idxA_sb = persist([128, NB * CA * 8], I16, "idxA_sb")
        idxB_sb = persist([128, NB * CB * 8], I16, "idxB_sb")
        dinv_sb = persist([128, NB], F32, "dinv_sb")
        bias_sb = persist([3, cfg.h1], BF16, "bias_sb")
        ident_sb = persist([128, 128], F32, "ident_sb")
        w_sb = {
            "W1c": persist([128, KCH * cfg.h1], BF16, "w1_sb"),
            "Wrc": persist([128, KCH * cfg.h1], BF16, "wr_sb"),
            "Wxc": persist([128, KCH * cfg.h2], BF16, "wx_sb"),
        }
        ht_sb = [persist([128, PER], F32, f"ht{k}_sb") for k in range(KCH)]

        nc.sync.dma_start(idxA_sb[:, :], idxA_d[:, :])
        nc.sync.dma_start(idxB_sb[:, :], idxB_d[:, :])
        nc.sync.dma_start(dinv_sb[:, :], dinv_d[:, :])
        nc.sync.dma_start(bias_sb[:, :], bias_d[:, :])
        nc.sync.dma_start(ident_sb[:, :], ident_d[:, :])
        for nm, dt_ in w_d.items():
            nc.sync.dma_start(w_sb[nm][:, :], dt_[:, :])

        with (
            tc.tile_pool(name="lhs", bufs=4) as lhs_pool,
            tc.tile_pool(name="gout", bufs=3) as gout_pool,
            tc.tile_pool(name="mst", bufs=2) as m_pool,
            tc.tile_pool(name="ga", bufs=2) as ga_pool,
            tc.tile_pool(name="gb", bufs=2) as gb_pool,
            tc.tile_pool(name="hrow", bufs=2) as hrow_pool,
            tc.tile_pool(name="psg", bufs=2, space="PSUM") as psg_pool,
            tc.tile_pool(name="psa", bufs=2, space="PSUM") as psa_pool,
            tc.tile_pool(name="pst", bufs=2, space="PSUM") as pst_pool,
            tc.tile_pool(name="dram", bufs=1, space="DRAM") as dram_pool,
        ):
            regA = nc.gpsimd.to_reg(CA * 128)
            regB = nc.gpsimd.to_reg(CB * 128)
            for l in range(6):
                ch = CH[l]
                w_t = w_sb[WNAME[l]]
                agin = dram_pool.tile([PER, ch], BF16, name=f"agin{l}", tag=f"agin{l}")
                agout = dram_pool.tile([NPAD + 1, ch], BF16, name=f"agout{l}", tag=f"agout{l}")
                # ---- GEMM phase: G' rows for our PER nodes
                for b in range(NB):
                    pg = psg_pool.tile([128, ch], F32, name="pg", tag="pg")
                    for k in range(KCH):
                        lhs = lhs_pool.tile([128, 128], BF16, name="lhs", tag="lhs")
                        if l == 0:
                            nc.sync.dma_start(
                                lhs[:, :],
                                xT_d[k * 128 : (k + 1) * 128, b * 128 : (b + 1) * 128],
                            )
                        else:
                            nc.vector.tensor_copy(
                                lhs[:, :], ht_sb[k][:, b * 128 : (b + 1) * 128]
                            )
                        nc.tensor.matmul(
                            pg[:, :],
                            lhsT=lhs[:, :],
                            rhs=w_t[:, k * ch : (k + 1) * ch],
                            start=(k == 0),
                            stop=(k == KCH - 1),
                        )
                    go = gout_pool.tile([128, ch], BF16, name="go", tag="go")
                    nc.scalar.mul(go[:, :], pg[:, :], dinv_sb[:, b : b + 1])
                    nc.sync.dma_start(agin[b * 128 : (b + 1) * 128, :], go[:, :])
                # ---- AllGather + bias row
                nc.gpsimd.collective_compute(
                    "AllGather",
                    mybir.AluOpType.bypass,
                    replica_groups=rg,
                    ins=[agin[:, :].opt()],
                    outs=[agout[0:NPAD, :].opt()],
                )
                nc.sync.dma_start(
                    agout[NPAD : NPAD + 1, :], bias_sb[BIDX[l] : BIDX[l] + 1, 0:ch]
                )
                # ---- aggregation phase
                for b in range(NB):
                    m_sb = m_pool.tile([128, CT * 128], BF16, name="m_sb", tag="m")
                    nc.sync.dma_start(
                        m_sb[:, :], Mst_d[:, b * CT * 128 : (b + 1) * CT * 128]
                    )
                    ga = ga_pool.tile([128, CA, ch], BF16, name="ga", tag="ga")
                    nc.gpsimd.dma_gather(
                        ga[:, :, :],
                        agout[0 : p.wa, :],
                        idxA_sb[:, b * CA * 8 : (b + 1) * CA * 8],
                        CA * 128,
                        regA,
                        ch,
                        single_packet=False,
                    )
                    gb = gb_pool.tile([128, CB, ch], BF16, name="gb", tag="gb")
                    nc.gpsimd.dma_gather(
                        gb[:, :, :],
                        agout[p.b0 : NPAD + 1, :],
                        idxB_sb[:, b * CB * 8 : (b + 1) * CB * 8],
                        CB * 128,
                        regB,
                        ch,
                        single_packet=False,
                    )
                    pa = psa_pool.tile([128, ch], F32, name="pa", tag="pa")
                    for c in range(CT):
                        rhs = ga[:, c, :] if c < CA else gb[:, c - CA, :]
                        nc.tensor.matmul(
                            pa[:, :],
                            lhsT=m_sb[:, c * 128 : (c + 1) * 128],
                            rhs=rhs,
                            start=(c == 0),
                            stop=(c == CT - 1),
                        )
                    if l == 5:
                        o = hrow_pool.tile([128, ch], F32, name="hrow", tag="hrow")
                        nc.scalar.mul(o[:, :], pa[:, :], dinv_sb[:, b : b + 1])
                        nc.sync.dma_start(out_d[b * 128 : (b + 1) * 128, :], o[:, :])
                    else:
                        hrow = hrow_pool.tile([128, ch], F32, name="hrow", tag="hrow")
                        nc.scalar.activation(
                            hrow[:, :],
                            pa[:, :],
                            mybir.ActivationFunctionType.Relu,
                            bias=0.0,
                            scale=dinv_sb[:, b : b + 1],
                        )
                        pt = pst_pool.tile([128, 512], F32, name="pt", tag="pt")
                        for k in range(KCH):
                            nc.tensor.transpose(
                                pt[:, k * 128 : (k + 1) * 128],
                                hrow[:, k * 128 : (k + 1) * 128],
                                ident_sb[:, :],
                            )
                            dstp = ht_sb[k][:, b * 128 : (b + 1) * 128]
                            if RES[l]:
                                nc.vector.tensor_add(
                                    dstp, pt[:, k * 128 : (k + 1) * 128], dstp
                                )
                            else:
                                nc.vector.tensor_copy(
                                    dstp, pt[:, k * 128 : (k + 1) * 128]
                                )
        for f in reversed(frees):
            f()
    nc.compile()
    return nc


def host_inputs(p: Prepped, x, W1, b1, Wr, br, Wx, bx):
    """Build per-core in_maps from the model inputs."""
    cfg = p.cfg
    KCH = cfg.in_ch // 128
    xp = np.zeros((cfg.npad, cfg.in_ch), dtype=np.float32)
    xp[p.newid] = np.asarray(x, dtype=np.float32)

    def wchunks(W, cho):
        W = np.asarray(W, dtype=np.float32)
        # [128, KCH*cho] with [p, k*cho+o] = W[k*128+p, o]
        return (
            W.reshape(KCH, 128, cho).transpose(1, 0, 2).reshape(128, KCH * cho)
        ).astype(NP_BF16)

    biases = np.zeros((3, cfg.h1), dtype=np.float32)
    biases[0, : cfg.h1] = np.asarray(b1)
    biases[1, : cfg.h1] = np.asarray(br)
    biases[2, : cfg.h2] = np.asarray(bx)
    biases = biases.astype(NP_BF16)
    W1c, Wrc = wchunks(W1, cfg.h1), wchunks(Wr, cfg.h1)
    Wxc = wchunks(Wx, cfg.h2)
    in_maps = []
    for c in range(cfg.cores):
        pc = p.per_core[c]
        xT = xp[c * cfg.per : (c + 1) * cfg.per].T.astype(NP_BF16).copy()
        in_maps.append(
            dict(
                xT=xT,
                ident=np.eye(128, dtype=np.float32),
                Mst=pc["Mst"],
                idxA=pc["idxA"],
                idxB=pc["idxB"],
                dinv=pc["dinv"],
                biases=biases,
                W1c=W1c,
                Wrc=Wrc,
                Wxc=Wxc,
            )
        )
    return in_maps


_CACHE = {}


def _get_compiled(edge_key, edge_index, cfg=None):
    if edge_key in _CACHE:
        return _CACHE[edge_key]
    cfg = cfg or GCNConfig()
    p = preprocess(cfg, edge_index)
    nc = build_program(p)
    _CACHE[edge_key] = (p, nc)
    return p, nc


def kernel(x, edge_index, W1, b1, Wr, br, Wx, bx):
    import time as _time

    edge_index = np.asarray(edge_index)
    key = (edge_index.shape[1], int(edge_index[:, ::4097].sum()))
    p, nc = _get_compiled(key, edge_index)
    in_maps = host_inputs(p, x, W1, b1, Wr, br, Wx, bx)
    t0 = _time.time()
    res = run_bass_kernel_spmd(nc, in_maps, core_ids=list(range(p.cfg.cores)))
    kernel.last_wall_s = _time.time() - t0
    full = np.concatenate([r["out"] for r in res.results], axis=0)
    out = full[p.newid].astype(np.float32)
    kernel.last_results = res
    return out


# revision 21
# speedup vs baseline: 1261.2312x; 1261.2312x over previous
"""GCN encoder (6 GCNConv layers, shared residual block) on 8 trn2 NeuronCores.

Strategy:
  - Nodes are relabeled (load-balanced bin packing by in-degree) and padded to
    NPAD = CORES * NB * 128. Core c owns the contiguous row range
    [c*PER, (c+1)*PER)  (PER = NB*128), i.e. NB blocks of 128 dst nodes.
  - Per conv: each core computes G' = dinv * (H @ W) for its own rows (dense
    GEMM on the PE), casts to bf16, AllGathers G' so every core holds the full
    [NPAD, ch] table in local DRAM, then aggregates messages for its dst
    blocks: dma_gather pulls 128 source rows per chunk into SBUF in matmul
    layout, and a one-hot "scatter matrix" M (stationary, built on host)
    scatter-adds them into a [128 dst, ch] PSUM tile:
        psum = sum_c  M_c.T @ Grows_c          (PE matmul accumulation)
    Self-loops and the bias are folded in as extra edges (bias edge gathers a
    bias row appended to the G' table, with M column values 1/dinv_dst).
    Postscale by dinv_dst + ReLU is a single ScalarE activation; the residual
    add runs on VectorE against the transposed f32 H state (PE transposes).
  - dma_gather indices are int16, so the gather is split into two windows of
    the node table: window A = rows [0, 32768), window B = rows
    [NPAD+1-32768, NPAD+1). Edges with src in the overlap can go to either
    call; the host balances the two chunk lists per block.
"""

import sys

sys.path.insert(0, "/opt/trn_rl_repo")

from dataclasses import dataclass

import numpy as np

import concourse.bass as bass
import concourse.bacc as bacc
import concourse.mybir as mybir
from concourse.bass_utils import run_bass_kernel_spmd
from concourse.tile import TileContext

BF16 = mybir.dt.bfloat16
F32 = mybir.dt.float32
I16 = mybir.dt.int16
NP_BF16 = mybir.dt.np(BF16)


@dataclass
class GCNConfig:
    n_nodes: int = 50000
    cores: int = 8
    nb: int = 49          # dst blocks per core
    in_ch: int = 512
    h1: int = 512
    h2: int = 256
    depth: int = 4        # residual reuses of Wr
    # derived
    @property
    def per(self):
        return self.nb * 128

    @property
    def npad(self):
        return self.cores * self.per

    @property
    def nblk(self):
        return self.cores * self.nb


@dataclass
class Prepped:
    cfg: GCNConfig
    ca: int               # chunks per block gathered from window A
    cb: int               # chunks per block gathered from window B
    wa: int               # window A row count (in_ap rows), idx < wa
    b0: int               # window B first row;  idx = src - b0
    newid: np.ndarray     # [n_nodes] -> padded id
    per_core: list        # per core dict: Mst, idxA, idxB, dinv
    host_ms: float = 0.0


def preprocess(cfg: GCNConfig, edge_index: np.ndarray) -> Prepped:
    import time

    t0 = time.time()
    N, NPAD, NBLK = cfg.n_nodes, cfg.npad, cfg.nblk
    src = np.asarray(edge_index[0], dtype=np.int64)
    dst = np.asarray(edge_index[1], dtype=np.int64)
    deg = np.bincount(dst, minlength=N).astype(np.int64) + 1  # + self loop
    dinv = (1.0 / np.sqrt(deg)).astype(np.float32)

    # ---- load-balanced node relabeling: snake-deal nodes (sorted by degree
    # desc) into NBLK blocks of <=128 slots; block sums come out ~equal.
    order = np.argsort(-deg, kind="stable")
    n_rounds = -(-N // NBLK)
    blk_of = np.empty(N, dtype=np.int64)
    slot_of = np.empty(N, dtype=np.int64)
    pos = np.arange(N)
    r = pos // NBLK
    j = pos % NBLK
    fwd = (r % 2) == 0
    b = np.where(fwd, j, NBLK - 1 - j)
    blk_of[order] = b
    slot_of[order] = r
    assert n_rounds <= 128
    newid = blk_of * 128 + slot_of
    # Reserve the last slot of each core's last block as a pad: every core
    # overwrites its agin row PER-1 with the bias vector, so those 8 table
    # rows hold b after the AllGather (bias edges gather row NPAD-1).
    nb_ = NBLK // cfg.cores
    reserved = [((c * nb_ + nb_ - 1) * 128 + 127) for c in range(cfg.cores)]
    occupied = np.full(NPAD, -1, dtype=np.int64)
    occupied[newid] = np.arange(N)
    free_ids = np.setdiff1d(np.where(occupied < 0)[0], np.array(reserved))
    fi = 0
    for rid in reserved:
        node = occupied[rid]
        if node >= 0:
            newid[node] = free_ids[fi]
            occupied[free_ids[fi]] = node
            occupied[rid] = -1
            fi += 1
    # dinv per padded id (pads -> 1.0)
    dinv_pad = np.ones(NPAD, dtype=np.float32)
    dinv_pad[newid] = dinv
    real_pad = np.zeros(NPAD, dtype=bool)
    real_pad[newid] = True

    # ---- edge lists in new ids, plus self loops
    es = newid[src]
    ed = newid[dst]
    es = np.concatenate([es, newid])          # self loops
    ed = np.concatenate([ed, newid])
    eb = ed >> 7                              # dst block
    eslot = ed & 127

    # sort edges by (block, src) so each block's list is src-ascending
    sort_key = eb * (NPAD + 1) + es
    so = np.argsort(sort_key, kind="stable")
    es, eb, eslot = es[so], eb[so], eslot[so]

    cnt = np.bincount(eb, minlength=NBLK)     # real+self edges per block
    tmax = int(cnt.max())
    # window geometry
    half = 32768
    wa = min(half, NPAD)                      # window A rows [0, wa)
    b0 = max(0, NPAD - half)                  # window B rows [b0, NPAD)
    n_low = np.bincount(eb[es < b0], minlength=NBLK) if b0 > 0 else np.zeros(NBLK, int)
    n_high = np.bincount(eb[es >= wa], minlength=NBLK) if wa < NPAD else np.zeros(NBLK, int)
    ct = -(-(tmax + 1) // 128)                # +1 bias edge
    ca_lo = -(-int(n_low.max()) // 128) if b0 > 0 else 0
    cb_lo = -(-(int(n_high.max()) + 1) // 128)
    while ca_lo + cb_lo > ct:
        ct += 1
    ca = min(max(ca_lo, ct - cb_lo, ct // 2), ct - cb_lo)
    cb = ct - ca
    assert ca * 128 >= n_low.max() and cb * 128 >= n_high.max() + 1

    # per-block split index into the src-sorted edge list
    starts = np.zeros(NBLK + 1, dtype=np.int64)
    np.cumsum(cnt, out=starts[1:])
    n_lowmid = np.bincount(eb[es < wa], minlength=NBLK)
    split = np.minimum(ca * 128, n_lowmid)    # first `split` edges -> call A
    nb_b = cnt - split                        # B real edges (bias appended after)
    assert (nb_b + 1 <= cb * 128).all()

    # position of each edge within its block
    epos = np.arange(len(es)) - starts[eb]
    in_a = epos < split[eb]
    posA = epos
    posB = epos - split[eb]

    # chunk index within block (A chunks [0,ca), B chunks [ca, ct)), edge slot
    echunk = np.where(in_a, posA >> 7, ca + (posB >> 7))
    erow = np.where(in_a, posA & 127, posB & 127)

    # ---- per-core arrays
    core_of = eb // cfg.nb
    bl = eb % cfg.nb                          # block local index
    per_core = []
    ca8, cb8 = ca * 8, cb * 8
    for c in range(cfg.cores):
        m = core_of == c
        ces, cbl, cch, cer, cslot = es[m], bl[m], echunk[m], erow[m], eslot[m]
        cina = in_a[m]
        # scatter matrix M: [128, nb*ct*128] ; M[erow, (bl*ct+ch)*128 + slot] = 1
        M = np.zeros((128, cfg.nb * ct * 128), dtype=np.float32)
        M[cer, (cbl * ct + cch) * 128 + cslot] = 1.0
        # idx lists: [nb, ca*128] / [nb, cb*128] (pad 0)
        idxA = np.zeros((cfg.nb, ca * 128), dtype=np.int64)
        idxB = np.zeros((cfg.nb, cb * 128), dtype=np.int64)
        idxA[cbl[cina], (cch[cina] * 128 + cer[cina])] = ces[cina]
        mB = ~cina
        idxB[cbl[mB], ((cch[mB] - ca) * 128 + cer[mB])] = ces[mB] - b0
        # bias edge per block: B position nb_b[block]
        gblk = c * cfg.nb + np.arange(cfg.nb)
        bpos = nb_b[gblk]
        bch, brow = ca + (bpos >> 7), bpos & 127
        idxB[np.arange(cfg.nb), (bch - ca) * 128 + brow] = (NPAD - 1 - b0)
        # M row for bias edge: 1/dinv over real slots of the block
        dv = dinv_pad.reshape(NBLK, 128)[gblk]           # [nb, 128]
        rl = real_pad.reshape(NBLK, 128)[gblk]
        bias_w = np.where(rl, 1.0 / dv, 0.0).astype(np.float32)
        cols = ((np.arange(cfg.nb) * ct + bch) * 128)[:, None] + np.arange(128)[None, :]
        M[brow[:, None].repeat(128, 1), cols] = bias_w
        assert idxA.min() >= 0 and idxA.max() < wa
        assert idxB.min() >= 0 and idxB.max() < NPAD - b0
        # wrap idx to [128, nb*ca8] int16 layout: idx i at [i%16, i//16], tiled x8
        def wrap(a, c8):
            w = a.reshape(cfg.nb, c8, 16).transpose(2, 0, 1).reshape(16, cfg.nb * c8)
            return np.tile(w, (8, 1)).astype(np.int16)
        per_core.append(
            dict(
                Mst=M.astype(NP_BF16),
                idxA=wrap(idxA, ca8),
                idxB=wrap(idxB, cb8),
                dinv=dinv_pad.reshape(NBLK, 128)[gblk].T.copy(),  # [128, nb]
            )
        )
    return Prepped(cfg, ca, cb, wa, b0, newid, per_core, host_ms=(time.time() - t0) * 1e3)


def build_program(p: Prepped, mode: str = "full", repeat: int = 1, shared_ag: bool = True, deep_bufs: bool = False, m_fp8: bool = False):
    cfg = p.cfg
    NB, PER, NPAD = cfg.nb, cfg.per, cfg.npad
    CA, CB, CT = p.ca, p.cb, p.ca + p.cb
    CH = [cfg.h1] * 5 + [cfg.h2]
    RELU = [True] * 5 + [False]
    RES = [False] + [True] * 4 + [False]
    WNAME = ["W1c"] + ["Wrc"] * 4 + ["Wxc"]
    BIDX = [0] + [1] * 4 + [2]
    KCH = cfg.in_ch // 128  # = 4

    nc = bacc.Bacc(num_devices=cfg.cores)
    xT_d = nc.dram_tensor("xT", [cfg.in_ch, PER], BF16, kind="ExternalInput")
    MDT = mybir.dt.float8e4 if m_fp8 else BF16
    Mst_d = nc.dram_tensor("Mst", [128, NB * CT * 128], MDT, kind="ExternalInput")
    idxA_d = nc.dram_tensor("idxA", [128, NB * CA * 8], I16, kind="ExternalInput")
    idxB_d = nc.dram_tensor("idxB", [128, NB * CB * 8], I16, kind="ExternalInput")
    dinv_d = nc.dram_tensor("dinv", [128, NB], F32, kind="ExternalInput")
    ident_d = nc.dram_tensor("ident", [128, 128], F32, kind="ExternalInput")
    bias_d = nc.dram_tensor("biases", [3, cfg.h1], BF16, kind="ExternalInput")
    w_d = {
        "W1c": nc.dram_tensor("W1c", [128, KCH * cfg.h1], BF16, kind="ExternalInput"),
        "Wrc": nc.dram_tensor("Wrc", [128, KCH * cfg.h1], BF16, kind="ExternalInput"),
        "Wxc": nc.dram_tensor("Wxc", [128, KCH * cfg.h2], BF16, kind="ExternalInput"),
    }
    out_d = nc.dram_tensor("out", [PER, cfg.h2], F32, kind="ExternalOutput")
    rg = [list(range(cfg.cores))]

    with TileContext(nc) as tc:
        frees = []

        def persist(shape, dtype, name, space="SBUF"):
            t, f = tc.tile(shape, dtype, name=name, space=space)
            frees.append(f)
            return t

        idxA_sb = persist([128, NB * CA * 8], I16, "idxA_sb")
        idxB_sb = persist([128, NB * CB * 8], I16, "idxB_sb")
        dinv_sb = persist([128, NB], F32, "dinv_sb")
        bias_sb = persist([3, cfg.h1], BF16, "bias_sb")
        ident_sb = persist([128, 128], F32, "ident_sb")
        w_sb = {
            "W1c": persist([128, KCH * cfg.h1], BF16, "w1_sb"),
            "Wrc": persist([128, KCH * cfg.h1], BF16, "wr_sb"),
            "Wxc": persist([128, KCH * cfg.h2], BF16, "wx_sb"),
        }
        ht_sb = [persist([128, PER], F32, f"ht{k}_sb") for k in range(KCH)]

        nc.sync.dma_start(idxA_sb[:, :], idxA_d[:, :])
        nc.sync.dma_start(idxB_sb[:, :], idxB_d[:, :])
        nc.sync.dma_start(dinv_sb[:, :], dinv_d[:, :])
        nc.sync.dma_start(bias_sb[:, :], bias_d[:, :])
        nc.sync.dma_start(ident_sb[:, :], ident_d[:, :])
        for nm, dt_ in w_d.items():
            nc.sync.dma_start(w_sb[nm][:, :], dt_[:, :])

        if mode == "trivial":
            triv = persist([128, cfg.h2], F32, "triv")
            nc.sync.dma_start(triv[:, :], xT_d[0:128, 0 : 2 * cfg.h2].bitcast(F32))
            with (
                tc.tile_pool(name="tps", bufs=1, space="PSUM") as tps,
                tc.tile_pool(name="tsb", bufs=1) as tsb,
            ):
                pp = tps.tile([128, 128], F32, name="pp")
                bb = tsb.tile([128, 128], BF16, name="bb")
                nc.vector.tensor_copy(bb[:, :], ident_sb[:, :])
                nc.tensor.matmul(pp[:, :], lhsT=bb[:, :], rhs=bb[:, :], start=True, stop=True)
                nc.scalar.mul(triv[:, 0:128], pp[:, :], 1.0)
                nc.gpsimd.memset(bb[:, :], 0.0)
            for b in range(NB):
                nc.sync.dma_start(out_d[b * 128 : (b + 1) * 128, :], triv[:, :])
            for f in reversed(frees):
                f()
            nc.compile()
            return nc
        with (
            tc.tile_pool(name="lhs", bufs=4) as lhs_pool,
            tc.tile_pool(name="gout", bufs=3) as gout_pool,
            tc.tile_pool(name="mst", bufs=3 if deep_bufs else 2) as m_pool,
            tc.tile_pool(name="ga", bufs=3 if deep_bufs else 2) as ga_pool,
            tc.tile_pool(name="gb", bufs=3 if deep_bufs else 2) as gb_pool,
            tc.tile_pool(name="hrow", bufs=2) as hrow_pool,
            tc.tile_pool(name="psg", bufs=2, space="PSUM") as psg_pool,
            tc.tile_pool(name="psa", bufs=2, space="PSUM") as psa_pool,
            tc.tile_pool(name="pst", bufs=2, space="PSUM") as pst_pool,
            tc.tile_pool(name="dram", bufs=1, space="DRAM") as dram_pool,
        ):
            regA = nc.gpsimd.to_reg(CA * 128)
            regB = nc.gpsimd.to_reg(CB * 128)
            for l in [li for _ in range(repeat) for li in range(6)]:
                ch = CH[l]
                w_t = w_sb[WNAME[l]]
                agin = dram_pool.tile([PER, ch], BF16, name=f"agin{l}", tag=f"agin{l}")
                agout = dram_pool.tile(
                    [NPAD, ch], BF16, name=f"agout{l}", tag=f"agout{l}",
                    addr_space="Shared" if shared_ag else "Local",
                )
                # ---- GEMM phase: G' rows for our PER nodes
                for b in range(NB):
                    pg = psg_pool.tile([128, ch], F32, name="pg", tag="pg")
                    for k in range(KCH):
                        lhs = lhs_pool.tile([128, 128], BF16, name="lhs", tag="lhs")
                        if l == 0:
                            nc.sync.dma_start(
                                lhs[:, :],
                                xT_d[k * 128 : (k + 1) * 128, b * 128 : (b + 1) * 128],
                            )
                        else:
                            nc.vector.tensor_copy(
                                lhs[:, :], ht_sb[k][:, b * 128 : (b + 1) * 128]
                            )
                        nc.tensor.matmul(
                            pg[:, :],
                            lhsT=lhs[:, :],
                            rhs=w_t[:, k * ch : (k + 1) * ch],
                            start=(k == 0),
                            stop=(k == KCH - 1),
                        )
                    go = gout_pool.tile([128, ch], BF16, name="go", tag="go")
                    nc.scalar.mul(go[:, :], pg[:, :], dinv_sb[:, b : b + 1])
                    nc.sync.dma_start(agin[b * 128 : (b + 1) * 128, :], go[:, :])
                # bias vector into our last (reserved-pad) row, pre-AG
                nc.sync.dma_start(
                    agin[PER - 1 : PER, :], bias_sb[BIDX[l] : BIDX[l] + 1, 0:ch]
                )
                # ---- AllGather
                if mode not in ("noag", "gemm"):
                    nc.gpsimd.collective_compute(
                        "AllGather",
                        mybir.AluOpType.bypass,
                        replica_groups=rg,
                        ins=[agin[:, :].opt()],
                        outs=[agout[:, :].opt()],
                    )
                # ---- aggregation phase
                if mode == "gemm":
                    for b in range(NB):
                        pg2 = psa_pool.tile([128, ch], F32, name="pa", tag="pa")
                        nc.tensor.matmul(pg2[:, :], lhsT=w_t[:, 0:128], rhs=w_t[:, 0:ch], start=True, stop=True)
                        if l == 5:
                            o = hrow_pool.tile([128, ch], F32, name="hrow", tag="hrow")
                            nc.scalar.mul(o[:, :], pg2[:, :], dinv_sb[:, b : b + 1])
                            nc.sync.dma_start(out_d[b * 128 : (b + 1) * 128, :], o[:, :])
                        else:
                            hrow = hrow_pool.tile([128, ch], F32, name="hrow", tag="hrow")
                            nc.scalar.activation(hrow[:, :], pg2[:, :], mybir.ActivationFunctionType.Relu, bias=0.0, scale=dinv_sb[:, b : b + 1])
                            pt = pst_pool.tile([128, 512], F32, name="pt", tag="pt")
                            for k in range(KCH):
                                nc.tensor.transpose(pt[:, k * 128 : (k + 1) * 128], hrow[:, k * 128 : (k + 1) * 128], ident_sb[:, :])
                                dstp = ht_sb[k][:, b * 128 : (b + 1) * 128]
                                if RES[l]:
                                    nc.vector.tensor_add(dstp, pt[:, k * 128 : (k + 1) * 128], dstp)
                                else:
                                    nc.vector.tensor_copy(dstp, pt[:, k * 128 : (k + 1) * 128])
                    continue
                for b in range(NB):
                    m_sb = m_pool.tile([128, CT * 128], MDT, name="m_sb", tag="m")
                    nc.sync.dma_start(
                        m_sb[:, :], Mst_d[:, b * CT * 128 : (b + 1) * CT * 128]
                    )
                    ga = ga_pool.tile([128, CA, ch], BF16, name="ga", tag="ga")
                    gb = gb_pool.tile([128, CB, ch], BF16, name="gb", tag="gb")
                    if mode == "nogather" and l == 0 and b < 2:
                        nc.vector.memset(ga[:, :, :], 0.02)
                        nc.vector.memset(gb[:, :, :], 0.02)
                    if mode != "nogather":
                      nc.gpsimd.dma_gather(
                        ga[:, :, :],
                        agout[0 : p.wa, :],
                        idxA_sb[:, b * CA * 8 : (b + 1) * CA * 8],
                        CA * 128,
                        regA,
                        ch,
                        single_packet=False,
                      )
                      nc.gpsimd.dma_gather(
                        gb[:, :, :],
                        agout[p.b0 : NPAD, :],
                        idxB_sb[:, b * CB * 8 : (b + 1) * CB * 8],
                        CB * 128,
                        regB,
                        ch,
                        single_packet=False,
                      )
                    pa = psa_pool.tile([128, ch], F32, name="pa", tag="pa")
                    for c in range(CT):
                        rhs = ga[:, c, :] if c < CA else gb[:, c - CA, :]
                        nc.tensor.matmul(
                            pa[:, :],
                            lhsT=m_sb[:, c * 128 : (c + 1) * 128],
                            rhs=rhs,
                            start=(c == 0),
                            stop=(c == CT - 1),
                        )
                    if l == 5:
                        o = hrow_pool.tile([128, ch], F32, name="hrow", tag="hrow")
                        nc.scalar.mul(o[:, :], pa[:, :], dinv_sb[:, b : b + 1])
                        nc.sync.dma_start(out_d[b * 128 : (b + 1) * 128, :], o[:, :])
                    else:
                        hrow = hrow_pool.tile([128, ch], F32, name="hrow", tag="hrow")
                        nc.scalar.activation(
                            hrow[:, :],
                            pa[:, :],
                            mybir.ActivationFunctionType.Relu,
                            bias=0.0,
                            scale=dinv_sb[:, b : b + 1],
                        )
                        pt = pst_pool.tile([128, 512], F32, name="pt", tag="pt")
                        for k in range(KCH):
                            nc.tensor.transpose(
                                pt[:, k * 128 : (k + 1) * 128],
                                hrow[:, k * 128 : (k + 1) * 128],
                                ident_sb[:, :],
                            )
                            dstp = ht_sb[k][:, b * 128 : (b + 1) * 128]
                            if RES[l]:
                                nc.vector.tensor_add(
                                    dstp, pt[:, k * 128 : (k + 1) * 128], dstp
                                )
                            else:
                                nc.vector.tensor_copy(
                                    dstp, pt[:, k * 128 : (k + 1) * 128]
                                )
        for f in reversed(frees):
            f()
    nc.compile()
    return nc


def host_inputs(p: Prepped, x, W1, b1, Wr, br, Wx, bx, m_fp8: bool = False):
    """Build per-core in_maps from the model inputs."""
    cfg = p.cfg
    KCH = cfg.in_ch // 128
    xp = np.zeros((cfg.npad, cfg.in_ch), dtype=np.float32)
    xp[p.newid] = np.asarray(x, dtype=np.float32)

    def wchunks(W, cho):
        W = np.asarray(W, dtype=np.float32)
        # [128, KCH*cho] with [p, k*cho+o] = W[k*128+p, o]
        return (
            W.reshape(KCH, 128, cho).transpose(1, 0, 2).reshape(128, KCH * cho)
        ).astype(NP_BF16)

    biases = np.zeros((3, cfg.h1), dtype=np.float32)
    biases[0, : cfg.h1] = np.asarray(b1)
    biases[1, : cfg.h1] = np.asarray(br)
    biases[2, : cfg.h2] = np.asarray(bx)
    biases = biases.astype(NP_BF16)
    W1c, Wrc = wchunks(W1, cfg.h1), wchunks(Wr, cfg.h1)
    Wxc = wchunks(Wx, cfg.h2)
    in_maps = []
    for c in range(cfg.cores):
        pc = p.per_core[c]
        xT = xp[c * cfg.per : (c + 1) * cfg.per].T.astype(NP_BF16).copy()
        in_maps.append(
            dict(
                xT=xT,
                ident=np.eye(128, dtype=np.float32),
                Mst=pc["Mst"].astype(mybir.dt.np(mybir.dt.float8e4)) if m_fp8 else pc["Mst"],
                idxA=pc["idxA"],
                idxB=pc["idxB"],
                dinv=pc["dinv"],
                biases=biases,
                W1c=W1c,
                Wrc=Wrc,
                Wxc=Wxc,
            )
        )
    return in_maps


_CACHE = {}


def _get_compiled(edge_key, edge_index, cfg=None):
    if edge_key in _CACHE:
        return _CACHE[edge_key]
    cfg = cfg or GCNConfig()
    p = preprocess(cfg, edge_index)
    nc = build_program(p)
    _CACHE[edge_key] = (p, nc)
    return p, nc


def kernel(x, edge_index, W1, b1, Wr, br, Wx, bx):
    import time as _time

    edge_index = np.asarray(edge_index)
    key = (edge_index.shape[1], int(edge_index[:, ::4097].sum()))
    p, nc = _get_compiled(key, edge_index)
    in_maps = host_inputs(p, x, W1, b1, Wr, br, Wx, bx)
    t0 = _time.time()
    res = run_bass_kernel_spmd(nc, in_maps, core_ids=list(range(p.cfg.cores)))
    kernel.last_wall_s = _time.time() - t0
    full = np.concatenate([r["out"] for r in res.results], axis=0)
    out = full[p.newid].astype(np.float32)
    kernel.last_results = res
    return out


def timed_run(x, edge_index, W1, b1, Wr, br, Wx, bx, iters=8, profile_dir=None):
    edge_index = np.asarray(edge_index)
    key = (edge_index.shape[1], int(edge_index[:, ::4097].sum()))
    p, nc = _get_compiled(key, edge_index)
    in_maps = host_inputs(p, x, W1, b1, Wr, br, Wx, bx)
    return timed_exec(p, nc, in_maps, iters=iters, profile_dir=profile_dir)


def timed_exec(p, nc, in_maps, iters=8, profile_dir=None):
    """Steady-state execution timing: inputs resident on device, jit once,
    no donation, repeat. Returns (per-iter wall seconds list, output)."""
    import time as _time

    import jax
    from jax.sharding import Mesh, PartitionSpec
    from jax.experimental.shard_map import shard_map
    from concourse import bass2jax, mybir as _mb

    n_cores = p.cfg.cores
    bass2jax.install_neuronx_cc_hook()

    in_names, out_names, out_avals = [], [], []
    for alloc in nc.m.functions[0].allocations:
        if not isinstance(alloc, mybir.MemoryLocationSet):
            continue
        name = alloc.memorylocations[0].name
        if alloc.kind == "ExternalInput":
            if nc.partition_id_tensor is None or name != nc.partition_id_tensor.name:
                in_names.append(name)
        elif alloc.kind == "ExternalOutput":
            out_names.append(name)
            out_avals.append(
                jax.core.ShapedArray(tuple(alloc.tensor_shape), _mb.dt.np(alloc.dtype))
            )
    n_params = len(in_names)
    zero_outs = [np.zeros(a.shape, a.dtype) for a in out_avals]
    all_in = list(in_names) + list(out_names)
    if nc.partition_id_tensor is not None:
        all_in.append(nc.partition_id_tensor.name)

    def _body(*args):
        operands = list(args)
        if nc.partition_id_tensor is not None:
            operands.append(bass2jax.partition_id_tensor())
        outs = bass2jax._bass_exec_p.bind(
            *operands,
            out_avals=tuple(out_avals),
            in_names=tuple(all_in),
            out_names=tuple(out_names),
            lowering_input_output_aliases=(),
            sim_require_finite=True,
            sim_require_nnan=True,
            nc=nc,
        )
        return tuple(outs)

    devices = jax.devices()[:n_cores]
    mesh = Mesh(np.asarray(devices), ("core",))
    f = jax.jit(
        shard_map(
            _body,
            mesh=mesh,
            in_specs=(PartitionSpec("core"),) * (n_params + len(out_names)),
            out_specs=(PartitionSpec("core"),) * len(out_names),
            check_rep=False,
        ),
        keep_unused=True,
    )
    concat_in = [
        jax.device_put(
            np.concatenate([np.asarray(in_maps[c][nm]) for c in range(n_cores)], 0),
            jax.sharding.NamedSharding(mesh, PartitionSpec("core")),
        )
        for nm in in_names
    ] + [
        jax.device_put(
            np.concatenate([z] * n_cores, 0),
            jax.sharding.NamedSharding(mesh, PartitionSpec("core")),
        )
        for z in zero_outs
    ]
    # warmup + compile
    out = f(*concat_in)
    jax.block_until_ready(out)
    walls = []
    for _ in range(iters):
        t0 = _time.time()
        out = f(*concat_in)
        jax.block_until_ready(out)
        walls.append(_time.time() - t0)
    if profile_dir:
        with jax.profiler.trace(profile_dir):
            out = f(*concat_in)
            jax.block_until_ready(out)
    full = np.asarray(out[0]).reshape(n_cores * p.cfg.per, p.cfg.h2)
    return walls, full[p.newid].astype(np.float32)
